# revision 32
# baseline (speedup 1.0000x reference)
"""Trainium2 Bass kernel for nn_Decoder (6-layer transformer decoder).

Strategy: data-parallel over batch B=16 across 8 NeuronCores (2 sequences
per core), weights replicated. Per core everything is computed feature-major
(activations stored transposed, [features on partitions, tokens on free dim])
so every linear layer is a weight-stationary matmul and no on-device
transposes are needed.

v2 changes vs the fp32r baseline:
  - everything the PE touches is bf16 (weights, activations, P, V, the
    residual stream itself): bf16 matmuls stream at ~0.5 ns/row vs ~0.65
    for fp32r and their LDWEIGHTS fully hide behind the previous matmul.
  - bf16 residual stream unlocks the DVE 2x/4x fast modes for the
    layernorm element-wise chain and halves SBUF so y stays resident.
  - softmax denominators: ACT Ln reads row 64 of the AV psum directly
    (no DVE gather), one pair-batched Exp(-x) gives both heads'
    reciprocals, and a single [2,128]-ones matmul broadcasts the pair
    (half the broadcast matmuls, one normalize mul per pair).
  - attention-output copies (AV psum -> sbuf) moved to the idle GPSIMD
    engine; squares for LN variance moved to DVE 4x mode.
  - cross-attention K/V projections depend only on the encoder output y
    (layer-invariant, resident in SBUF): they are issued during the
    layernorm serial chains so the PE never idles there.
  - FFN processes 512-token chunks (half the instructions), FFN2
    accumulates m-outer so 3 PSUM banks suffice.

Host side transposes x/y/weights into these layouts (numpy), shards the
batch, and runs the single compiled Bass program SPMD on cores 0-7.
"""
import sys

if "/opt/trn_rl_repo" not in sys.path:
    sys.path.insert(0, "/opt/trn_rl_repo")

import ml_dtypes
import numpy as np

import concourse.bass as bass
import concourse.mybir as mybir
import concourse.tile as tile
from concourse import bacc
from concourse.bass_utils import run_bass_kernel_spmd

# The ACT-table placement pass maps Exp -> "exp_and_others" and Ln ->
# "natural_log", so a kernel using both thrashes ACT_TABLE_LOADs (~1.3us
# each) inside the softmax/LN chain.  Advertise Exp/Ln only from the
# combined "natural_log_exp_and_others" set (indices are preserved, so the
# emitted act_func_set_id still matches act_info.json) -> one load total.
_orig_get_act_tables = bacc.get_activation_tables


def _patched_get_act_tables(arch):
    tables = dict(_orig_get_act_tables(arch))
    exp = mybir.ActivationFunctionType.Exp
    ln = mybir.ActivationFunctionType.Ln
    if any(exp in f and ln in f for f in tables.values()):
        out = {}
        for name, fns in tables.items():
            if exp in fns and ln in fns:
                out[name] = fns
            else:
                out[name] = fns - {exp, ln}
        return out
    return tables


bacc.get_activation_tables = _patched_get_act_tables

_bf16 = ml_dtypes.bfloat16
F32 = mybir.dt.float32
F32R = mybir.dt.float32r
BF16 = mybir.dt.bfloat16
AF = mybir.ActivationFunctionType
ALU = mybir.AluOpType

L, H, D, DH, DFF = 6, 8, 512, 64, 2048
EPS = 1e-5
NCORES = 8
BLOC = 2            # sequences per core
S = 512             # tokens per sequence
TLOC = BLOC * S     # tokens per core
KT = D // 128       # 4 contraction k-tiles for D
MT = D // 128       # 4 output feature m-tiles
FCH = 256           # FFN token chunk
NH = TLOC // FCH    # 4 chunks


def _build(trivial_ln: bool, trivial_bias: bool):
    nc = bacc.Bacc("TRN2", target_bir_lowering=False, debug=False)

    xT = nc.dram_tensor("xT", [D, TLOC], F32R, kind="ExternalInput")
    xTb = nc.dram_tensor("xTb", [D, TLOC], BF16, kind="ExternalInput")
    yT = nc.dram_tensor("yT", [D, TLOC], BF16, kind="ExternalInput")
    wq1 = nc.dram_tensor("wq1", [L, D, D], BF16, kind="ExternalInput")
    wk1 = nc.dram_tensor("wk1", [L, D, D], BF16, kind="ExternalInput")
    wv1 = nc.dram_tensor("wv1", [L, D, D], BF16, kind="ExternalInput")
    wo1 = nc.dram_tensor("wo1", [L, D, D], BF16, kind="ExternalInput")
    wq2 = nc.dram_tensor("wq2", [L, D, D], BF16, kind="ExternalInput")
    wk2 = nc.dram_tensor("wk2", [L, D, D], BF16, kind="ExternalInput")
    wv2 = nc.dram_tensor("wv2", [L, D, D], BF16, kind="ExternalInput")
    wo2 = nc.dram_tensor("wo2", [L, D, D], BF16, kind="ExternalInput")
    w1 = nc.dram_tensor("w1", [L, D, DFF], BF16, kind="ExternalInput")
    w2 = nc.dram_tensor("w2", [L, DFF, D], BF16, kind="ExternalInput")
    oblkc = nc.dram_tensor("oblkc", [33, 128], F32R, kind="ExternalInput")
    if not trivial_ln:
        lng = nc.dram_tensor("lng", [3, L, D], F32, kind="ExternalInput")
        lnb = nc.dram_tensor("lnb", [3, L, D], F32, kind="ExternalInput")
    if not trivial_bias:
        bf1 = nc.dram_tensor("bf1", [L, DFF], F32, kind="ExternalInput")
        bf2 = nc.dram_tensor("bf2", [L, D], F32, kind="ExternalInput")
    outT = nc.dram_tensor("outT", [D, TLOC], F32, kind="ExternalOutput")

    from contextlib import ExitStack
    with ExitStack() as _ctx:
        tc = _ctx.enter_context(tile.TileContext(nc))
        _ctx.enter_context(nc.allow_low_precision(reason="bf16 pipeline"))

        def _pool(name, bufs, space="SBUF"):
            return _ctx.enter_context(tc.tile_pool(name=name, bufs=bufs, space=space))

        cst = _pool("cst", 1)
        hidp = _pool("hid", 5)       # f32r residual stream (LN outputs)
        hidb = _pool("hidb", 8)      # bf16 matmul copies of the stream
        sres = _pool("sres", 4)      # f32r pre-LN sums
        ybp = _pool("ybp", 4)        # resident y (bf16)
        kb2p = _pool("kb2", 4)       # persistent cross-K
        vvp = _pool("vv", 4)         # persistent self-V (augmented)
        vv2p = _pool("vv2", 8)       # persistent cross-V (augmented)
        qkp = _pool("qk", 4)
        ptp = _pool("pt", 5)
        osbp = _pool("osb", 3)
        oalp = _pool("oal", 4)
        stbp = _pool("stb", 3)       # bf16 LN stats (m, msq)
        stfp = _pool("stf", 4)       # f32 LN stats (var, lnv, rstd)
        outp = _pool("outp", 2)      # f32 final-output staging
        t1p = _pool("t1", 4)
        sqp = _pool("sq", 3)
        f1p = _pool("f1", 16)
        awp = _pool("aw", 1)         # [128, KT*512] attn weight matrices (4 tags)
        wkv2p = _pool("wkv2", 1)     # cross K/V weights (prefetched)
        w1p = _pool("w1p", 1)
        w2p = _pool("w2p", 1)
        b1p = _pool("b1", 16)
        tinyp = _pool("tiny", 8)
        psA = _pool("psA", 3, "PSUM")
        psB = _pool("psB", 3, "PSUM")
        psC = _pool("psC", 2, "PSUM")

        # ---------------- constants ----------------
        onesf = cst.tile([128, 128], F32, tag="onesf", name="onesf")
        nc.gpsimd.memset(onesf[:], 1.0)
        onesb = cst.tile([128, 128], BF16, tag="onesb", name="onesb")
        nc.vector.tensor_copy(onesb[:], onesf[:])
        onesr = cst.tile([128, 128], F32R, tag="onesr", name="onesr")
        nc.vector.tensor_copy(onesr[:], onesf[:])
        # pair-broadcast stationary: col j<64 <- row 0, col j>=64 <- row 32,
        # rows 1-31 zero (engine writes must start at partition 0/32/64/96,
        # so the pair's denominators live at rows 0 and 32).
        oblk = cst.tile([33, 128], F32R, tag="oblk", name="oblk")
        nc.sync.dma_start(oblk[:], oblkc.ap()[:, :])
        # persistent double-buffered denominator tiles, rows 1-31 stay zero.
        # rec is f32r: the reciprocal is a per-(token,head) scale, so its
        # rounding error is systematic — keep it full precision (the f32r
        # broadcast matmul runs at 1 cycle/row just like bf16).
        lnds, recs = [], []
        for i in range(2):
            t = cst.tile([33, 512], F32, tag=f"lnd{i}", name="lnd")
            nc.gpsimd.memset(t[:], 0.0)
            lnds.append(t)
            r = cst.tile([33, 512], F32R, tag=f"rec{i}", name="rec")
            nc.vector.tensor_copy(r[:], t[:])  # f32r memset is invalid ISA
            recs.append(r)
        # causal diag-block mask: keep (0) where t(p) <= q(j), else -1e30
        maskD = cst.tile([128, 128], F32, tag="maskD")
        nc.gpsimd.memset(maskD[:], 0.0)
        nc.gpsimd.affine_select(
            out=maskD[:], in_=maskD[:], compare_op=ALU.is_ge,
            fill=-1e30, base=0, pattern=[[1, 128]], channel_multiplier=-1,
        )
        epsb = cst.tile([128, 1], F32, tag="epsb", name="epsb")
        nc.gpsimd.memset(epsb[:], EPS)

        # ---------------- initial activation / y load ----------------
        xcur, xb = [], []
        for k in range(KT):
            t = hidb.tile([128, TLOC], BF16, tag="hidb", name="hidb")
            nc.sync.dma_start(t[:], xTb.ap()[k * 128:(k + 1) * 128, :])
            xb.append(t)
        for k in range(KT):
            t = hidp.tile([128, TLOC], F32R, tag="hid", name="hid")
            nc.sync.dma_start(t[:], xT.ap()[k * 128:(k + 1) * 128, :])
            xcur.append(t)
        yb = []
        for k in range(KT):
            t = ybp.tile([128, TLOC], BF16, tag="yb", name="yb")
            nc.sync.dma_start(t[:], yT.ap()[k * 128:(k + 1) * 128, :])
            yb.append(t)

        # persistent V tiles (augmented with an all-ones denominator col)
        def _new_vtile(pool, tag):
            va = pool.tile([128, H * 65], BF16, tag=tag, name=tag)
            nc.vector.tensor_copy(
                va[:].rearrange("p (h c) -> p h c", h=H)[:, :, 64:65],
                onesf[:, 0:H].rearrange("p (h o) -> p h o", o=1))
            return va

        vper = [_new_vtile(vvp, "vv") for _ in range(4)]
        vper2 = [_new_vtile(vv2p, "vv2") for _ in range(8)]  # [b*4+tt]
        # persistent cross-K, feature-major, both seqs
        kb2 = [kb2p.tile([128, TLOC], BF16, tag="kb2", name="kb2")
               for _ in range(MT)]

        def load_w(pool, dram, l, tag, q="sync", cols=D):
            """One DMA: [KT*128, cols] matrix -> [128, KT, cols] tile."""
            t = pool.tile([128, KT * cols], BF16, tag=tag, name=tag)
            eng = nc.sync if q == "sync" else nc.gpsimd
            eng.dma_start(
                t[:].rearrange("p (k d) -> p k d", k=KT),
                dram.ap()[l].rearrange("(k p) d -> p k d", k=KT))
            return t

        def wsl(t, k, mi=None):
            if mi is None:
                return t[:, k * D:(k + 1) * D]
            return t[:, k * D + mi * 128: k * D + (mi + 1) * 128]

        def ln_params(idx, l):
            if trivial_ln:
                return None, None
            gs, bs = [], []
            for k in range(KT):
                g = tinyp.tile([128, 1], F32, tag="lng", name="lng")
                nc.sync.dma_start(
                    g[:], lng.ap()[idx, l, k * 128:(k + 1) * 128].rearrange("p -> p 1"))
                b = tinyp.tile([128, 1], F32, tag="lnb", name="lnb")
                nc.sync.dma_start(
                    b[:], lnb.ap()[idx, l, k * 128:(k + 1) * 128].rearrange("p -> p 1"))
                gs.append(g)
                bs.append(b)
            return gs, bs

        def layer_norm(src, idx, l, last=False):
            """src: KT f32r tiles [128, TLOC]. Returns (out f32r, outb bf16)
            tile lists; for the last LN only the DMA to outT happens."""
            gs, bs = ln_params(idx, l)
            out = [hidp.tile([128, TLOC], F32R, tag="hid", name="hid")
                   for _ in range(KT)] if not last else None
            outb = [hidb.tile([128, TLOC], BF16, tag="hidb", name="hidb")
                    for _ in range(KT)] if not last else None
            for nh in range(2):
                cs = slice(nh * 512, (nh + 1) * 512)
                sum_ps = psB.tile([128, 512], F32, tag="sc", name="st")
                ssq_ps = psB.tile([128, 512], F32, tag="sc", name="st")
                for k in range(KT):
                    nc.tensor.matmul(sum_ps[:], onesr[:], src[k][:, cs],
                                     start=(k == 0), stop=(k == KT - 1))
                sq = []
                for k in range(KT):
                    s = sqp.tile([128, 512], BF16, tag="sq", name="sq")
                    nc.gpsimd.tensor_mul(s[:], src[k][:, cs], src[k][:, cs])
                    sq.append(s)
                for k in range(KT):
                    nc.tensor.matmul(ssq_ps[:], onesb[:], sq[k][:],
                                     start=(k == 0), stop=(k == KT - 1))
                m = stbp.tile([128, 512], BF16, tag="stb", name="stb")
                nc.vector.tensor_scalar_mul(m[:], sum_ps[:], 1.0 / D)
                msq = stbp.tile([128, 512], BF16, tag="stb", name="stb")
                nc.vector.tensor_mul(msq[:], m[:], m[:])
                var = stfp.tile([128, 512], F32, tag="stf", name="stf")
                nc.vector.scalar_tensor_tensor(
                    var[:], ssq_ps[:], 1.0 / D, msq[:], ALU.mult, ALU.subtract)
                lnv = stfp.tile([128, 512], F32, tag="stf", name="stf")
                nc.scalar.activation(lnv[:], var[:], AF.Ln, bias=epsb[:])
                # rstd is a per-token scale: systematic error, keep f32
                rstd = stfp.tile([128, 512], F32, tag="stf", name="stf")
                nc.scalar.activation(rstd[:], lnv[:], AF.Exp, scale=-0.5)
                for k in range(KT):
                    t1 = t1p.tile([128, 512], F32, tag="t1", name="t1")
                    nc.vector.tensor_sub(t1[:], src[k][:, cs], m[:])
                    if last:
                        # final output in f32 (bf16 would alone cost ~4e-3)
                        outf = outp.tile([128, 512], F32, tag="outf", name="outf")
                        nc.vector.tensor_mul(outf[:], t1[:], rstd[:])
                        if not trivial_ln:
                            nc.vector.tensor_scalar(
                                outf[:], outf[:], gs[k][:], bs[k][:],
                                ALU.mult, ALU.add)
                        nc.sync.dma_start(
                            outT.ap()[k * 128:(k + 1) * 128, cs], outf[:])
                    elif trivial_ln:
                        nc.vector.tensor_mul(outb[k][:, cs], t1[:], rstd[:])
                        nc.vector.tensor_mul(out[k][:, cs], t1[:], rstd[:])
                    else:
                        t2 = t1p.tile([128, 512], F32, tag="t2", name="t2")
                        nc.vector.tensor_mul(t2[:], t1[:], rstd[:])
                        nc.vector.tensor_scalar(
                            outb[k][:, cs], t2[:], gs[k][:], bs[k][:],
                            ALU.mult, ALU.add)
                        nc.vector.tensor_scalar(
                            out[k][:, cs], t2[:], gs[k][:], bs[k][:],
                            ALU.mult, ALU.add)
            return out, outb

        def project_k2(wk_t, b):
            """cross K for seq b into the persistent kb2 tiles."""
            bs = slice(b * S, (b + 1) * S)
            for mi in range(MT):
                ps = psA.tile([128, 512], F32, tag="mm", name="mm")
                for k in range(KT):
                    nc.tensor.matmul(ps[:], wsl(wk_t, k, mi), yb[k][:, bs],
                                     start=(k == 0), stop=(k == KT - 1))
                nc.vector.tensor_copy(kb2[mi][:, bs], ps[:])

        def project_v2(wv_t, b):
            """cross V (token-major, augmented) for seq b."""
            for tt in range(4):
                t0 = b * S + tt * 128
                ps = psA.tile([128, 512], F32, tag="mm", name="mm")
                for k in range(KT):
                    nc.tensor.matmul(ps[:], yb[k][:, t0:t0 + 128], wsl(wv_t, k),
                                     start=(k == 0), stop=(k == KT - 1))
                va = vper2[b * 4 + tt]
                nc.vector.tensor_copy(
                    va[:].rearrange("p (h c) -> p h c", h=H)[:, :, 0:64],
                    ps[:].rearrange("p (h c) -> p h c", h=H))

        def attention(qsrc, l, wq_t, wk_t, wv_t, wo_t, causal, resid_src):
            """Full MHA block. qsrc: KT bf16 tiles. Self-attn (causal) builds
            K/V from qsrc; cross (not causal) uses precomputed kb2/vper2.
            Returns s = attn_out + resid (KT bf16 sres tiles)."""
            oall = [oalp.tile([128, TLOC], BF16, tag="oal", name="oal")
                    for _ in range(MT)]
            for b in range(BLOC):
                bs = slice(b * S, (b + 1) * S)
                # Q (and for self-attn K) projections, feature-major [D, S]
                qb, kb = [], []
                proj = [(qb, wq_t, "q")]
                if causal:
                    proj.append((kb, wk_t, "k"))
                for (dst, wt, tg) in proj:
                    for mi in range(MT):
                        ps = psA.tile([128, 512], F32, tag="mm", name="mm")
                        for k in range(KT):
                            nc.tensor.matmul(
                                ps[:], wsl(wt, k, mi), qsrc[k][:, bs],
                                start=(k == 0), stop=(k == KT - 1))
                        o = qkp.tile([128, S], BF16, tag=tg, name=tg)
                        nc.vector.tensor_copy(o[:], ps[:])
                        dst.append(o)
                if causal:
                    # V token-major augmented: [128 tok, 8*(64+1)]
                    vb = []
                    for tt in range(4):
                        ps = psA.tile([128, 512], F32, tag="mm", name="mm")
                        for k in range(KT):
                            nc.tensor.matmul(
                                ps[:], qsrc[k][:, b * S + tt * 128: b * S + tt * 128 + 128],
                                wsl(wv_t, k),
                                start=(k == 0), stop=(k == KT - 1))
                        va = vper[tt]
                        nc.vector.tensor_copy(
                            va[:].rearrange("p (h c) -> p h c", h=H)[:, :, 0:64],
                            ps[:].rearrange("p (h c) -> p h c", h=H))
                        vb.append(va)
                    kstat = [(kb[mi], 0) for mi in range(MT)]
                else:
                    vb = vper2[b * 4: b * 4 + 4]
                    kstat = [(kb2[mi], b * S) for mi in range(MT)]

                # heads in pairs; the normalize runs one pair behind so the
                # in-order PE stream never waits on the reciprocal chain.
                pending = []

                def flush_norm(item):
                    mi2, rec, osb = item
                    bc = psB.tile([128, 512], F32, tag="sc", name="sc")
                    nc.tensor.matmul(bc[:], oblk[:], rec[0:33, :],
                                     start=True, stop=True)
                    nc.vector.tensor_mul(oall[mi2][:, bs], bc[:], osb[:])

                for hg in range(H // 2):
                    if len(pending) > 1:
                        flush_norm(pending.pop(0))
                    lnd = lnds[hg % 2]
                    rec = recs[hg % 2]
                    osb = osbp.tile([128, 512], BF16, tag="osb", name="osb")
                    for hh in range(2):
                        h = hg * 2 + hh
                        hb = hh * 64
                        kt_, koff = kstat[hg]
                        pts = []
                        for tt in range(4):
                            n0 = tt * 128 if causal else 0
                            sc_ps = psB.tile([128, 512], F32, tag="sc", name="sc")
                            nc.tensor.matmul(
                                sc_ps[:, n0:512],
                                kt_[hb:hb + 64, koff + tt * 128: koff + (tt + 1) * 128],
                                qb[hg][hb:hb + 64, n0:512],
                                start=True, stop=True)
                            ptt = ptp.tile([128, 512], BF16, tag="pt", name="pt")
                            if causal:
                                nc.vector.tensor_add(
                                    sc_ps[:, n0:n0 + 128], sc_ps[:, n0:n0 + 128],
                                    maskD[:])
                            nc.scalar.activation(
                                ptt[:, n0:512], sc_ps[:, n0:512], AF.Exp,
                                scale=0.125)
                            pts.append(ptt)
                        av = psC.tile([65, 512], F32, tag="av", name="av")
                        for tt in range(4):
                            n0 = tt * 128 if causal else 0
                            nc.tensor.matmul(
                                av[:, n0:512],
                                vb[tt][:, h * 65: h * 65 + 65],
                                pts[tt][:, n0:512],
                                start=(tt == 0), stop=(tt == 3))
                        # denominator ln straight off the psum row; the pair's
                        # logs land at rows 0 / 32 (legal partition bases)
                        nc.scalar.activation(
                            lnd[hh * 32:hh * 32 + 1, :], av[64:65, :], AF.Ln)
                        nc.vector.tensor_copy(
                            osb[hb:hb + 64, :], av[0:64, :])
                    # rows 1-31 of lnd are permanently 0 -> exp gives 1.0,
                    # weighted by the zero rows of oblk in the broadcast.
                    nc.scalar.activation(rec[0:33, :], lnd[0:33, :], AF.Exp,
                                         scale=-1.0)
                    pending.append((hg, rec, osb))
                while pending:
                    flush_norm(pending.pop(0))
            # output projection + residual
            s_out = [sres.tile([128, TLOC], F32R, tag="sres", name="sres")
                     for _ in range(MT)]
            for mi in range(MT):
                for nh in range(2):
                    cs = slice(nh * 512, (nh + 1) * 512)
                    ps = psA.tile([128, 512], F32, tag="mm", name="mm")
                    for k in range(KT):
                        nc.tensor.matmul(
                            ps[:], wsl(wo_t, k, mi), oall[k][:, cs],
                            start=(k == 0), stop=(k == KT - 1))
                    nc.vector.tensor_add(
                        s_out[mi][:, cs], ps[:], resid_src[mi][:, cs])
            return s_out

        # cross K/V weights for layer 0
        wk2_t = load_w(wkv2p, wk2, 0, "wk2")
        wv2_t = load_w(wkv2p, wv2, 0, "wv2")

        # ================= layer loop =================
        for l in range(L):
            wq1_t = load_w(awp, wq1, l, "awq")
            wk1_t = load_w(awp, wk1, l, "awk")
            wv1_t = load_w(awp, wv1, l, "awv")
            wo1_t = load_w(awp, wo1, l, "awo")
            # wq2/wo2 reuse wq1/wo1's buffers (freed once self-attn has
            # consumed them), halving resident attn-weight SBUF
            wq2_t = load_w(awp, wq2, l, "awq")
            wo2_t = load_w(awp, wo2, l, "awo")
            w1_t = load_w(w1p, w1, l, "w1", q="gpsimd", cols=DFF)  # [128, 4*2048]
            w2t = w2p.tile([128, (DFF // 128) * D], BF16, tag="w2", name="w2")
            nc.sync.dma_start(
                w2t[:].rearrange("p (k d) -> p k d", k=DFF // 128),
                w2.ap()[l].rearrange("(k p) d -> p k d", k=DFF // 128))
            if not trivial_bias:
                b1c, b2c = [], []
                for i in range(DFF // 128):
                    t = b1p.tile([128, 1], F32, tag="b1c", name="b1c")
                    nc.sync.dma_start(
                        t[:], bf1.ap()[l, i * 128:(i + 1) * 128].rearrange("p -> p 1"))
                    b1c.append(t)
                for i in range(MT):
                    t = tinyp.tile([128, 1], F32, tag="b2c", name="b2c")
                    nc.sync.dma_start(
                        t[:], bf2.ap()[l, i * 128:(i + 1) * 128].rearrange("p -> p 1"))
                    b2c.append(t)

            # ---- masked self-attention + LN1 ----
            s1 = attention(xb, l, wq1_t, wk1_t, wv1_t, wo1_t, True, xcur)
            h1, h1b = layer_norm(s1, 0, l)
            # cross V (and for l=0, K) projections fill the LN1 stall
            if l == 0:
                project_k2(wk2_t, 0)
                project_k2(wk2_t, 1)
            project_v2(wv2_t, 0)
            project_v2(wv2_t, 1)
            # ---- cross-attention + LN2 ----
            s2 = attention(h1b, l, wq2_t, None, None, wo2_t, False, h1)
            if l + 1 < L:
                wk2_t = load_w(wkv2p, wk2, l + 1, "wk2", q="gpsimd")
                wv2_t = load_w(wkv2p, wv2, l + 1, "wv2", q="gpsimd")
            h2, h2b = layer_norm(s2, 1, l)
            if l + 1 < L:
                project_k2(wk2_t, 0)   # fills the LN2 stall
            # ---- FFN ----
            s3 = [sres.tile([128, TLOC], F32R, tag="sres", name="sres")
                  for _ in range(MT)]
            for ch in range(NH):
                cs = slice(ch * FCH, (ch + 1) * FCH)
                f1t = []
                for m in range(DFF // 128):
                    ps = psA.tile([128, FCH], F32, tag="mm", name="mm")
                    for k in range(KT):
                        nc.tensor.matmul(
                            ps[:], w1_t[:, k * DFF + m * 128: k * DFF + (m + 1) * 128],
                            h2b[k][:, cs], start=(k == 0), stop=(k == KT - 1))
                    f = f1p.tile([128, FCH], BF16, tag="f1", name="f1")
                    nc.scalar.activation(
                        f[:], ps[:], AF.Relu,
                        bias=0.0 if trivial_bias else b1c[m][:])
                    f1t.append(f)
                # FFN2: s3 = f1 @ W2 + bf2 + h2   (m-outer, k accumulation)
                for m in range(MT):
                    fps = psA.tile([128, FCH], F32, tag="mm", name="mm")
                    for k in range(DFF // 128):
                        nc.tensor.matmul(
                            fps[:], w2t[:, k * D + m * 128: k * D + (m + 1) * 128],
                            f1t[k][:], start=(k == 0), stop=(k == DFF // 128 - 1))
                    if trivial_bias:
                        nc.vector.tensor_add(s3[m][:, cs], fps[:], h2[m][:, cs])
                    else:
                        nc.vector.scalar_tensor_tensor(
                            s3[m][:, cs], fps[:], b2c[m][:], h2[m][:, cs],
                            ALU.add, ALU.add)
            xcur, xb = layer_norm(s3, 2, l, last=(l == L - 1))
            if l + 1 < L:
                project_k2(wk2_t, 1)   # fills the LN3 stall

    nc.compile()
    return nc


_NC_CACHE = {}


def _get_nc(trivial_ln, trivial_bias):
    key = (trivial_ln, trivial_bias)
    if key not in _NC_CACHE:
        _NC_CACHE[key] = _build(trivial_ln, trivial_bias)
    return _NC_CACHE[key]


def _prep_inputs(inputs):
    f = np.float32
    x = np.asarray(inputs["x"], f)
    y = np.asarray(inputs["y"], f)

    def fm(w):  # [L, H, D, DH] -> [L, D, H*DH] bf16
        return np.ascontiguousarray(
            np.asarray(w, f).transpose(0, 2, 1, 3).reshape(L, D, H * DH)
            .astype(_bf16))

    def b16(w):
        return np.ascontiguousarray(np.asarray(w, f).astype(_bf16))

    shared = {
        "wq1": fm(inputs["Wq1"]), "wk1": fm(inputs["Wk1"]), "wv1": fm(inputs["Wv1"]),
        "wo1": b16(inputs["Wo1"]),
        "wq2": fm(inputs["Wq2"]), "wk2": fm(inputs["Wk2"]), "wv2": fm(inputs["Wv2"]),
        "wo2": b16(inputs["Wo2"]),
        "w1": b16(inputs["W1"]),
        "w2": b16(inputs["W2"]),
    }
    oblk = np.zeros((33, 128), f)
    oblk[0, 0:64] = 1.0
    oblk[32, 64:128] = 1.0
    shared["oblkc"] = np.ascontiguousarray(oblk)
    lng = np.stack([inputs["ln1_g"], inputs["ln2_g"], inputs["ln3_g"]]).astype(f)
    lnb = np.stack([inputs["ln1_b"], inputs["ln2_b"], inputs["ln3_b"]]).astype(f)
    bf1 = np.asarray(inputs["bf1"], f)
    bf2 = np.asarray(inputs["bf2"], f)
    trivial_ln = bool(np.all(lng == 1.0) and np.all(lnb == 0.0))
    trivial_bias = bool(np.all(bf1 == 0.0) and np.all(bf2 == 0.0))
    if not trivial_ln:
        shared["lng"] = np.ascontiguousarray(lng)
        shared["lnb"] = np.ascontiguousarray(lnb)
    if not trivial_bias:
        shared["bf1"] = np.ascontiguousarray(bf1)
        shared["bf2"] = np.ascontiguousarray(bf2)

    in_maps = []
    for c in range(NCORES):
        xc = x[c * BLOC:(c + 1) * BLOC].reshape(TLOC, D).T
        yc = y[c * BLOC:(c + 1) * BLOC].reshape(TLOC, D).T
        m = dict(shared)
        m["xT"] = np.ascontiguousarray(xc)
        m["xTb"] = np.ascontiguousarray(xc.astype(_bf16))
        m["yT"] = np.ascontiguousarray(yc.astype(_bf16))
        in_maps.append(m)
    return in_maps, trivial_ln, trivial_bias, x.shape


def run(inputs, trace=False, tmpdir=None):
    in_maps, trivial_ln, trivial_bias, xshape = _prep_inputs(inputs)
    nc = _get_nc(trivial_ln, trivial_bias)
    res = run_bass_kernel_spmd(
        nc, in_maps, list(range(NCORES)), trace=trace, tmpdir=tmpdir)
    B = xshape[0]
    out = np.empty((B, S, D), np.float32)
    for c in range(NCORES):
        out[c * BLOC:(c + 1) * BLOC] = (
            res.results[c]["outT"].astype(np.float32).T.reshape(BLOC, S, D))
    return out, res


def kernel(**inputs) -> np.ndarray:
    out, _ = run(inputs)
    return out
